# revision 2
# baseline (speedup 1.0000x reference)
"""BallLoss Trainium2 kernel (8-core data-parallel SPMD), v2.

loss = sum_{i,j} relu(d_i - d_ij),  d_ij = ||e_i - c_j||, d_i = d_{i,label_i}
     = sum_i [ C*d_i - sum_j min(d_ij, d_i) ]

Per-core (rows sharded along N across 8 cores, centers replicated):

  - PE:   p[i,j] = c2_j - 2*e_i.c_j via an augmented bf16 matmul
          (lhsT = [e_i; 1; 1], K=66; rhs = [-2c; c2_hi; c2_lo]).
  - Two per-tile pipelines, mixed to balance the ACT and DVE engines
    (measured: ACT runs 2x on bf16 input but 1x from fp32 PSUM; DVE
    tensor_scalar is 4x on bf16 SBUF, 1x with accum or PSUM input):
      S-tile (2 of 3): DVE evacuates PSUM with fused
            x = min(p + e2_i, d2_i)        [1x from PSUM, bf16 out]
          then ACT sqrt+accum on bf16:
            macc[:,t] = sum_j sqrt(x) = sum_j min(d_ij, d_i)   [2x]
      A-tile (1 of 3): ACT sqrt-evacuates PSUM
            dist = sqrt(p + e2_i)          [1x from PSUM]
          DVE min in-place (4x), then ACT Identity+accum (2x):
            macc[:,t] = sum_j min(dist, d_i)
  - d_i:  exact-ish from per-tile indirect-DMA gathers of c[label] in
          bf16: d2_i = sum_d (e_id - c_{lab_i,d})^2, d_i = sqrt(d2_i).
  - loss rows: C*d_i - macc, then on-chip scalar reduce per core.

Scheduling: work is emitted in 8-tile groups (loads + gathers + per-row
precompute + that group's main tiles) with explicit ordering deps that
keep the gather-gated d2 chain behind the previous group's main DVE ops
(the scheduler's DMA model underestimates indirect-gather latency).

Host: shards inputs, provides e / e^T layouts, bf16 casts and constant
ones-rows (layout prep only), casts labels to int32, sums the 8 per-core
scalars.
"""

from contextlib import ExitStack

import ml_dtypes
import numpy as np

import concourse.bass as bass
import concourse.tile as tile
from concourse import bacc, mybir
from concourse.bass_utils import run_bass_kernel_spmd

F32 = mybir.dt.float32
BF16 = mybir.dt.bfloat16
I32 = mybir.dt.int32
AF = mybir.ActivationFunctionType
OP = mybir.AluOpType
AX = mybir.AxisListType

N, C, D = 65536, 2048, 64
NCORES = 8
NS = N // NCORES  # 8192 rows per core
P = 128           # partitions
T = NS // P       # 64 row-tiles per core
FD = 512          # fp32 psum bank free dim
NB = C // FD      # 4 matmuls per row-tile
G = 8             # row-tiles per precompute group
NG = T // G       # 8 groups

MM_DT = BF16
KA = D + 2        # 64 e dims + c2 hi/lo ones rows


def _is_a_tile(t):
    # 1-of-3 tiles take the ACT-heavy path; offset so tile 0 (which
    # gates the pipeline start) is an S-tile.
    return t % 3 == 2


def _body(tc, out, eT, enat, labT, cT, cnat):
    nc = tc.nc
    with ExitStack() as ctx:
        const = ctx.enter_context(tc.tile_pool(name="const", bufs=1))

        eTa = const.tile([KA, NS], MM_DT)   # [66, 8192]
        chat = const.tile([KA, C], MM_DT)   # [66, 2048]
        craw = const.tile([D, C], F32)
        csq = const.tile([D, C], F32)
        ensb = const.tile([P, T * D], BF16)   # e natural, tile-major
        clab = const.tile([P, T * D], BF16)   # gathered centers per row
        scrb = const.tile([P, T * D], BF16)   # scratch squares
        labsb = const.tile([P, T], I32)
        ones = const.tile([P, 1], F32)
        e2 = const.tile([P, T], F32)
        d2 = const.tile([P, T], F32)
        dall = const.tile([P, T], F32)
        macc = const.tile([P, T], F32)
        rowtot = const.tile([P, 1], F32)
        outsb = const.tile([1, 1], F32)

        # labels first: the gpsimd gather stream is gated only on this DMA
        nc.sync.dma_start(labsb[:], labT)
        nc.vector.memset(ones[:], 1.0)

        mm_ctx = tc.tile_pool(name="mm", bufs=2, space="PSUM")
        mm_pool = mm_ctx.__enter__()

        # chat build, pipelined per 512-col bank chunk.
        c2ps_full = mm_pool.tile([P, C], F32, name="ps", tag="ps")
        c2ps = c2ps_full[0:1, :]
        c2hi = const.tile([1, C], MM_DT)
        c2lo = const.tile([1, C], MM_DT)
        for k in range(NB):
            sl = slice(k * FD, (k + 1) * FD)
            nc.sync.dma_start(craw[:, sl], cT[:, sl])
            nc.vector.tensor_mul(csq[:, sl], craw[:, sl], craw[:, sl])
            nc.tensor.matmul(
                c2ps[:, sl], lhsT=ones[0:D, :], rhs=csq[:, sl],
                start=True, stop=True,
            )
            nc.vector.tensor_scalar_mul(chat[0:D, sl], craw[:, sl], -2.0)
            nc.vector.tensor_copy(c2hi[:, sl], c2ps[:, sl])
            c2lo_i = nc.vector.tensor_sub(
                c2lo[:, sl], c2ps[:, sl], c2hi[:, sl]
            )
            nc.sync.dma_start(chat[D:D + 1, sl], c2hi[:, sl])
            nc.sync.dma_start(chat[D + 1:KA, sl], c2lo[:, sl])

        dist_pool = ctx.enter_context(tc.tile_pool(name="dist", bufs=6))
        from concourse.tile import add_dep_helper

        z_insts = []
        bounds = [(0, G // 2), (G // 2, G)] + [
            (g * G, (g + 1) * G) for g in range(1, NG)
        ]
        for gi, (ts, te) in enumerate(bounds):
            cs, ce = ts * P, te * P
            fs, fe = ts * D, te * D
            nc.sync.dma_start(eTa[:, cs:ce], eT[:, cs:ce])
            nc.sync.dma_start(
                ensb[:, fs:fe].rearrange("p (t d) -> p t d", d=D),
                enat[cs:ce, :].rearrange("(t p) d -> p t d", p=P),
            )
            for t in range(ts, te):
                nc.gpsimd.indirect_dma_start(
                    out=clab[:, t * D:(t + 1) * D],
                    out_offset=None,
                    in_=cnat,
                    in_offset=bass.IndirectOffsetOnAxis(ap=labsb[:, t:t + 1], axis=0),
                )
            # per-row e2, d2, d
            nc.vector.tensor_mul(scrb[:, fs:fe], ensb[:, fs:fe], ensb[:, fs:fe])
            nc.vector.tensor_reduce(
                e2[:, ts:te], scrb[:, fs:fe].rearrange("p (t d) -> p t d", d=D),
                axis=AX.X, op=OP.add,
            )
            nc.gpsimd.tensor_sub(
                clab[:, fs:fe], ensb[:, fs:fe], clab[:, fs:fe]
            )
            sub_i = nc.vector.tensor_mul(
                scrb[:, fs:fe], clab[:, fs:fe], clab[:, fs:fe]
            )
            if gi >= 2:
                # keep the gather-gated d2 chain BEHIND the previous group's
                # main DVE ops in the scheduled stream
                add_dep_helper(sub_i.ins, z_insts[ts - 3].ins, sync=False,
                               reason="hold d2 chain behind prior group")
            elif gi == 1:
                add_dep_helper(sub_i.ins, z_insts[1].ins, sync=False,
                               reason="hold d2 chain behind prior group")
            else:
                add_dep_helper(sub_i.ins, c2lo_i.ins, sync=False,
                               reason="hold g0 d2 chain behind chat build")
            nc.vector.tensor_reduce(
                d2[:, ts:te], scrb[:, fs:fe].rearrange("p (t d) -> p t d", d=D),
                axis=AX.X, op=OP.add,
            )
            nc.scalar.activation(dall[:, ts:te], d2[:, ts:te], AF.Sqrt)

            # main tiles of this group
            for t in range(ts, te):
                ps = mm_pool.tile([P, C], F32, name="ps")
                lhsT = eTa[:, t * P:(t + 1) * P]
                for k in range(NB):
                    nc.tensor.matmul(
                        ps[:, k * FD:(k + 1) * FD],
                        lhsT=lhsT,
                        rhs=chat[:, k * FD:(k + 1) * FD],
                        start=True, stop=True,
                    )
                xz = dist_pool.tile([P, C], BF16, name="dist")
                if _is_a_tile(t):
                    # ACT sqrt-evac, DVE min 4x in-place, ACT id+accum
                    nc.scalar.activation(
                        xz[:], ps[:], AF.Sqrt,
                        bias=e2[:, t:t + 1], scale=1.0,
                    )
                    zi = nc.vector.tensor_scalar(
                        out=xz[:], in0=xz[:],
                        scalar1=dall[:, t:t + 1], scalar2=None,
                        op0=OP.min,
                    )
                    nc.scalar.activation(
                        xz[:], xz[:], AF.Identity,
                        accum_out=macc[:, t:t + 1],
                    )
                else:
                    # DVE evac with fused (p + e2) min d2, ACT sqrt+accum
                    zi = nc.vector.tensor_scalar(
                        out=xz[:], in0=ps[:],
                        scalar1=e2[:, t:t + 1], scalar2=d2[:, t:t + 1],
                        op0=OP.add, op1=OP.min,
                    )
                    nc.scalar.activation(
                        xz[:], xz[:], AF.Sqrt,
                        accum_out=macc[:, t:t + 1],
                    )
                z_insts.append(zi)

        mm_ctx.__exit__(None, None, None)

        # rowrelu[p,t] = C * dall - macc  -> reuse macc
        nc.vector.scalar_tensor_tensor(
            out=macc[:], in0=dall[:], scalar=float(C), in1=macc[:],
            op0=OP.mult, op1=OP.subtract,
        )

        # loss_partial = sum_{p,t} macc
        nc.vector.tensor_reduce(rowtot[:], macc[:], axis=AX.X, op=OP.add)
        with tc.tile_pool(name="fin", bufs=1, space="PSUM") as finp:
            fin = finp.tile([1, 1], F32)
            nc.tensor.matmul(fin[:], lhsT=rowtot[:], rhs=ones[:], start=True, stop=True)
            nc.scalar.copy(outsb[:], fin[:])
        nc.sync.dma_start(out, outsb[:])


_NC_CACHE = {}


def build_nc():
    if "nc" in _NC_CACHE:
        return _NC_CACHE["nc"]
    nc = bacc.Bacc(
        "TRN2", target_bir_lowering=False, debug=False, enable_asserts=False
    )
    eT = nc.dram_tensor("eT", [KA, NS], MM_DT, kind="ExternalInput").ap()
    enat = nc.dram_tensor("enat", [NS, D], BF16, kind="ExternalInput").ap()
    labT = nc.dram_tensor("labT", [P, T], I32, kind="ExternalInput").ap()
    cT = nc.dram_tensor("cT", [D, C], F32, kind="ExternalInput").ap()
    cnat = nc.dram_tensor("cnat", [C, D], BF16, kind="ExternalInput").ap()
    out = nc.dram_tensor("out", [1, 1], F32, kind="ExternalOutput").ap()
    with nc.allow_low_precision(reason="bf16 distance pipeline"):
        with tile.TileContext(nc) as tc:
            _body(tc, out, eT, enat, labT, cT, cnat)
    nc.compile()
    _NC_CACHE["nc"] = nc
    return nc


def make_in_maps(embeddings, centers, labels):
    e = np.ascontiguousarray(np.asarray(embeddings, dtype=np.float32))
    c = np.ascontiguousarray(np.asarray(centers, dtype=np.float32))
    lab = np.asarray(labels).astype(np.int32)
    assert e.shape == (N, D) and c.shape == (C, D) and lab.shape == (N,)
    cT = np.ascontiguousarray(c.T)
    cb = c.astype(ml_dtypes.bfloat16)
    in_maps = []
    for core in range(NCORES):
        es = e[core * NS:(core + 1) * NS]
        ls = lab[core * NS:(core + 1) * NS]
        eT66 = np.ones((KA, NS), np.float32)
        eT66[0:D] = es.T
        eT66 = eT66.astype(ml_dtypes.bfloat16)
        in_maps.append({
            "eT": eT66,
            "enat": np.ascontiguousarray(es.astype(ml_dtypes.bfloat16)),
            "labT": np.ascontiguousarray(ls.reshape(T, P).T),
            "cT": cT,
            "cnat": cb,
        })
    return in_maps


def run(embeddings, centers, labels, **kw):
    nc = build_nc()
    in_maps = make_in_maps(embeddings, centers, labels)
    res = run_bass_kernel_spmd(nc, in_maps, core_ids=list(range(NCORES)), **kw)
    total = float(sum(float(r["out"][0, 0]) for r in res.results))
    return np.float32(total), res


def kernel(embeddings, centers, labels):
    val, _ = run(embeddings, centers, labels)
    return val


# revision 5
# speedup vs baseline: 1.1751x; 1.1751x over previous
"""BallLoss Trainium2 kernel (8-core data-parallel SPMD), v3.

loss = sum_{i,j} relu(d_i - d_ij),  d_ij = ||e_i - c_j||, d_i = d_{i,label_i}
     = sum_i [ C*d_i - sum_j min(d_ij, d_i) ]

Per-core (rows sharded along N across 8 cores, centers replicated):

  - PE:   p[i,j] = c2_j - 2*e_i.c_j via an augmented bf16 matmul, K=65:
          lhsT = [-2*e_i; 1] (host supplies the -2 scale on the e side),
          rhs  = [c^T; c2] (c^T DMA'd straight from the host in bf16,
          c2 computed on-device into psum partition 64 via a ones-matmul
          and copied across to chat row 64 on the same partition).
  - ACT:  dist[i,j] = sqrt(p[i,j] + e2_i) (bias = e2 per partition),
          PSUM -> SBUF bf16, one op per [128, 2048] row-tile.
  - DVE:  min(dist, d_i) in-place at the 4x bf16 rate, then for most
          tiles (H) a 2x tensor_tensor add into a persistent bf16
          accumulator z_acc[128, 2048] (the loss only needs the GRAND
          sum of min, so per-row sums are unnecessary); a few tiles (A)
          instead go ACT Identity+accum -> macc to balance the two
          engines (ACT ~1.86-2.08us/tile vs DVE min+add ~1.98us/tile).
  - d_i:  from per-tile indirect-DMA gathers of c[label] in bf16:
          d2_i = sum_d (e_id - c_{lab_i,d})^2, d_i = sqrt(d2_i).
  - final: sum_i C*d_i - sum(macc) - sum(z_acc), reduced on-chip.

Scheduling: all input DMAs issued up front; work is emitted in 8-tile
groups (gathers + per-row precompute + main tiles) with explicit
ordering deps that keep the gather-gated d2 chain behind the previous
group's main DVE ops (the scheduler's DMA model underestimates
indirect-gather latency).

Host: shards inputs, provides layouts/casts only (e^T scaled by -2 in
bf16, ones row, bf16 copies of e and c, labels as int32), sums the 8
per-core scalars.
"""

from contextlib import ExitStack

import ml_dtypes
import numpy as np

import concourse.bass as bass
import concourse.tile as tile
from concourse import bacc, mybir
from concourse.bass_utils import run_bass_kernel_spmd

F32 = mybir.dt.float32
BF16 = mybir.dt.bfloat16
I32 = mybir.dt.int32
AF = mybir.ActivationFunctionType
OP = mybir.AluOpType
AX = mybir.AxisListType

N, C, D = 65536, 2048, 64
NCORES = 8
NS = N // NCORES  # 8192 rows per core
P = 128           # partitions
T = NS // P       # 64 row-tiles per core
FD = 512          # fp32 psum bank free dim
NB = C // FD      # 4 matmuls per row-tile
G = 8             # row-tiles per precompute group
NG = T // G       # 8 groups

MM_DT = BF16
KA = D + 1        # 64 e dims + c2 ones row

# tiles that take the ACT-heavy path (ACT id+accum instead of the DVE
# z_acc add) to balance the engines
A_TILES = frozenset({4, 13, 22, 31, 40, 49, 58})


def _body(tc, out, eT, enat, labT, cTb, cnat):
    nc = tc.nc
    with ExitStack() as ctx:
        const = ctx.enter_context(tc.tile_pool(name="const", bufs=1))

        eTa = const.tile([KA, NS], MM_DT)    # [65, 8192]
        chat = const.tile([KA, C], MM_DT)    # [65, 2048]: c^T rows + c2
        csqb = const.tile([D, C], BF16)
        ensb = const.tile([P, T * D], BF16)  # e natural, tile-major
        clab = const.tile([P, T * D], BF16)  # gathered centers per row
        scrb = const.tile([P, T * D], BF16)  # scratch squares
        labsb = const.tile([P, T], I32)
        ones = const.tile([P, 1], BF16)
        onesf = const.tile([P, 1], F32)
        zaccs = [const.tile([P, C], BF16, name=f"zacc{i}") for i in range(4)]
        e2 = const.tile([P, T], F32)
        d2 = const.tile([P, T], F32)
        dall = const.tile([P, T], F32)
        macc = const.tile([P, T], F32)
        rowtot = const.tile([P, 1], F32)
        zrow = const.tile([P, 1], F32)
        outsb = const.tile([1, 1], F32)

        # labels first: the gpsimd gather stream is gated only on this DMA
        nc.sync.dma_start(labsb[:], labT)
        # c^T lands straight into the matmul rhs rows 0..63
        nc.sync.dma_start(chat[0:D, :], cTb)
        nc.vector.memset(ones[:], 1.0)
        nc.vector.memset(onesf[:], 1.0)
        for za in zaccs:
            nc.vector.memset(za[:], 0.0)
        nc.vector.memset(macc[:], 0.0)
        # all e-side loads issued up front (per-group slices keep the
        # tile framework's subtile deps intact)
        for g in range(NG):
            cs, ce = g * G * P, (g + 1) * G * P
            fs, fe = g * G * D, (g + 1) * G * D
            nc.sync.dma_start(eTa[:, cs:ce], eT[:, cs:ce])
            nc.sync.dma_start(
                ensb[:, fs:fe].rearrange("p (t d) -> p t d", d=D),
                enat[cs:ce, :].rearrange("(t p) d -> p t d", p=P),
            )

        mm_ctx = tc.tile_pool(name="mm", bufs=2, space="PSUM")
        mm_pool = mm_ctx.__enter__()

        # c2 row: csq = (c^T)^2, ones-matmul column sum into psum
        # partition 64, copy across to chat row 64 (same partition).
        c2ps_full = mm_pool.tile([P, C], F32, name="ps", tag="ps")
        for k in range(NB):
            sl = slice(k * FD, (k + 1) * FD)
            nc.vector.tensor_mul(csqb[:, sl], chat[0:D, sl], chat[0:D, sl])
            nc.tensor.matmul(
                c2ps_full[64:65, sl], lhsT=ones[0:D, :], rhs=csqb[:, sl],
                start=True, stop=True,
            )
            c2_i = nc.vector.tensor_copy(chat[D:KA, sl], c2ps_full[64:65, sl])

        dist_pool = ctx.enter_context(tc.tile_pool(name="dist", bufs=6))
        from concourse.tile import add_dep_helper

        z_insts = []
        nh = 0
        bounds = [(0, G // 2), (G // 2, G)] + [
            (g * G, (g + 1) * G) for g in range(1, NG)
        ]
        for gi, (ts, te) in enumerate(bounds):
            fs, fe = ts * D, te * D
            for t in range(ts, te):
                nc.gpsimd.indirect_dma_start(
                    out=clab[:, t * D:(t + 1) * D],
                    out_offset=None,
                    in_=cnat,
                    in_offset=bass.IndirectOffsetOnAxis(ap=labsb[:, t:t + 1], axis=0),
                )
            # per-row e2, d2, d
            nc.vector.tensor_mul(scrb[:, fs:fe], ensb[:, fs:fe], ensb[:, fs:fe])
            nc.vector.tensor_reduce(
                e2[:, ts:te], scrb[:, fs:fe].rearrange("p (t d) -> p t d", d=D),
                axis=AX.X, op=OP.add,
            )
            nc.gpsimd.tensor_sub(
                clab[:, fs:fe], ensb[:, fs:fe], clab[:, fs:fe]
            )
            sub_i = nc.vector.tensor_mul(
                scrb[:, fs:fe], clab[:, fs:fe], clab[:, fs:fe]
            )
            if gi >= 2:
                # keep the gather-gated d2 chain BEHIND the previous group's
                # main DVE ops in the scheduled stream
                add_dep_helper(sub_i.ins, z_insts[ts - 3].ins, sync=False,
                               reason="hold d2 chain behind prior group")
            elif gi == 1:
                add_dep_helper(sub_i.ins, z_insts[1].ins, sync=False,
                               reason="hold d2 chain behind prior group")
            else:
                add_dep_helper(sub_i.ins, c2_i.ins, sync=False,
                               reason="hold g0 d2 chain behind chat build")
            nc.vector.tensor_reduce(
                d2[:, ts:te], scrb[:, fs:fe].rearrange("p (t d) -> p t d", d=D),
                axis=AX.X, op=OP.add,
            )
            nc.scalar.activation(dall[:, ts:te], d2[:, ts:te], AF.Sqrt)

            # main tiles of this group
            for t in range(ts, te):
                ps = mm_pool.tile([P, C], F32, name="ps")
                lhsT = eTa[:, t * P:(t + 1) * P]
                for k in range(NB):
                    nc.tensor.matmul(
                        ps[:, k * FD:(k + 1) * FD],
                        lhsT=lhsT,
                        rhs=chat[:, k * FD:(k + 1) * FD],
                        start=True, stop=True,
                    )
                xz = dist_pool.tile([P, C], BF16, name="dist")
                nc.scalar.activation(
                    xz[:], ps[:], AF.Sqrt,
                    bias=e2[:, t:t + 1], scale=1.0,
                )
                zi = nc.vector.tensor_scalar(
                    out=xz[:], in0=xz[:],
                    scalar1=dall[:, t:t + 1], scalar2=dall[:, t:t + 1],
                    op0=OP.min, op1=OP.subtract,
                )
                z_insts.append(zi)
                if t in A_TILES:
                    nc.scalar.activation(
                        xz[:], xz[:], AF.Identity,
                        accum_out=macc[:, t:t + 1],
                    )
                else:
                    za = zaccs[nh % 4]
                    nc.vector.tensor_add(za[:], za[:], xz[:])
                    nh += 1

        mm_ctx.__exit__(None, None, None)

        # loss rows = -(sum_j macc + sum_j sum-of-zaccs); merge accs pairwise
        nc.vector.tensor_add(zaccs[0][:], zaccs[0][:], zaccs[1][:])
        nc.vector.tensor_add(zaccs[2][:], zaccs[2][:], zaccs[3][:])
        nc.vector.tensor_add(zaccs[0][:], zaccs[0][:], zaccs[2][:])
        nc.vector.tensor_reduce(rowtot[:], macc[:], axis=AX.X, op=OP.add)
        nc.vector.tensor_reduce(zrow[:], zaccs[0][:], axis=AX.X, op=OP.add)
        nc.vector.tensor_add(rowtot[:], rowtot[:], zrow[:])
        nc.vector.tensor_scalar_mul(rowtot[:], rowtot[:], -1.0)
        with tc.tile_pool(name="fin", bufs=1, space="PSUM") as finp:
            fin = finp.tile([1, 1], F32)
            nc.tensor.matmul(fin[:], lhsT=rowtot[:], rhs=onesf[:],
                             start=True, stop=True)
            nc.scalar.copy(outsb[:], fin[:])
        nc.sync.dma_start(out, outsb[:])


_NC_CACHE = {}


def build_nc():
    if "nc" in _NC_CACHE:
        return _NC_CACHE["nc"]
    nc = bacc.Bacc(
        "TRN2", target_bir_lowering=False, debug=False, enable_asserts=False
    )
    eT = nc.dram_tensor("eT", [KA, NS], MM_DT, kind="ExternalInput").ap()
    enat = nc.dram_tensor("enat", [NS, D], BF16, kind="ExternalInput").ap()
    labT = nc.dram_tensor("labT", [P, T], I32, kind="ExternalInput").ap()
    cTb = nc.dram_tensor("cTb", [D, C], BF16, kind="ExternalInput").ap()
    cnat = nc.dram_tensor("cnat", [C, D], BF16, kind="ExternalInput").ap()
    out = nc.dram_tensor("out", [1, 1], F32, kind="ExternalOutput").ap()
    with nc.allow_low_precision(reason="bf16 distance pipeline"):
        with tile.TileContext(nc) as tc:
            _body(tc, out, eT, enat, labT, cTb, cnat)
    nc.compile()
    _NC_CACHE["nc"] = nc
    return nc


def make_in_maps(embeddings, centers, labels):
    e = np.ascontiguousarray(np.asarray(embeddings, dtype=np.float32))
    c = np.ascontiguousarray(np.asarray(centers, dtype=np.float32))
    lab = np.asarray(labels).astype(np.int32)
    assert e.shape == (N, D) and c.shape == (C, D) and lab.shape == (N,)
    cTb = np.ascontiguousarray(c.T).astype(ml_dtypes.bfloat16)
    cb = c.astype(ml_dtypes.bfloat16)
    in_maps = []
    for core in range(NCORES):
        es = e[core * NS:(core + 1) * NS]
        ls = lab[core * NS:(core + 1) * NS]
        eT65 = np.ones((KA, NS), np.float32)
        eT65[0:D] = -2.0 * es.T
        eT65 = eT65.astype(ml_dtypes.bfloat16)
        in_maps.append({
            "eT": eT65,
            "enat": np.ascontiguousarray(es.astype(ml_dtypes.bfloat16)),
            "labT": np.ascontiguousarray(ls.reshape(T, P).T),
            "cTb": cTb,
            "cnat": cb,
        })
    return in_maps


def run(embeddings, centers, labels, **kw):
    nc = build_nc()
    in_maps = make_in_maps(embeddings, centers, labels)
    res = run_bass_kernel_spmd(nc, in_maps, core_ids=list(range(NCORES)), **kw)
    total = float(sum(float(r["out"][0, 0]) for r in res.results))
    return np.float32(total), res


def kernel(embeddings, centers, labels):
    val, _ = run(embeddings, centers, labels)
    return val


# revision 6
# speedup vs baseline: 1.1959x; 1.0177x over previous
"""BallLoss Trainium2 kernel (8-core data-parallel SPMD), v3.

loss = sum_{i,j} relu(d_i - d_ij),  d_ij = ||e_i - c_j||, d_i = d_{i,label_i}
     = sum_i [ C*d_i - sum_j min(d_ij, d_i) ]

Per-core (rows sharded along N across 8 cores, centers replicated):

  - PE:   p[i,j] = c2_j - 2*e_i.c_j via an augmented bf16 matmul, K=65:
          lhsT = [-2*e_i; 1] (host supplies the -2 scale on the e side),
          rhs  = [c^T; c2] (c^T DMA'd straight from the host in bf16,
          c2 computed on-device into psum partition 64 via a ones-matmul
          and copied across to chat row 64 on the same partition).
  - ACT:  dist[i,j] = sqrt(p[i,j] + e2_i) (bias = e2 per partition),
          PSUM -> SBUF bf16, one op per [128, 2048] row-tile.
  - DVE:  min(dist, d_i) in-place at the 4x bf16 rate, then for most
          tiles (H) a 2x tensor_tensor add into a persistent bf16
          accumulator z_acc[128, 2048] (the loss only needs the GRAND
          sum of min, so per-row sums are unnecessary); a few tiles (A)
          instead go ACT Identity+accum -> macc to balance the two
          engines (ACT ~1.86-2.08us/tile vs DVE min+add ~1.98us/tile).
  - d_i:  from per-tile indirect-DMA gathers of c[label] in bf16:
          d2_i = sum_d (e_id - c_{lab_i,d})^2, d_i = sqrt(d2_i).
  - final: sum_i C*d_i - sum(macc) - sum(z_acc), reduced on-chip.

Scheduling: all input DMAs issued up front; work is emitted in 8-tile
groups (gathers + per-row precompute + main tiles) with explicit
ordering deps that keep the gather-gated d2 chain behind the previous
group's main DVE ops (the scheduler's DMA model underestimates
indirect-gather latency).

Host: shards inputs, provides layouts/casts only (e^T scaled by -2 in
bf16, ones row, bf16 copies of e and c, labels as int32), sums the 8
per-core scalars.
"""

from contextlib import ExitStack

import ml_dtypes
import numpy as np

import concourse.bass as bass
import concourse.tile as tile
from concourse import bacc, mybir
from concourse.bass_utils import run_bass_kernel_spmd

F32 = mybir.dt.float32
BF16 = mybir.dt.bfloat16
I32 = mybir.dt.int32
AF = mybir.ActivationFunctionType
OP = mybir.AluOpType
AX = mybir.AxisListType

N, C, D = 65536, 2048, 64
NCORES = 8
NS = N // NCORES  # 8192 rows per core
P = 128           # partitions
T = NS // P       # 64 row-tiles per core
FD = 512          # fp32 psum bank free dim
NB = C // FD      # 4 matmuls per row-tile
G = 8             # row-tiles per precompute group
NG = T // G       # 8 groups

MM_DT = BF16
KA = D + 1        # 64 e dims + c2 ones row

# tiles that take the ACT-heavy path (ACT id+accum instead of the DVE
# z_acc add) to balance the engines
A_TILES = frozenset({4, 11, 18, 25, 32, 39, 46, 53, 60})


def _body(tc, out, eT, enat, labT, cTb, cnat):
    nc = tc.nc
    with ExitStack() as ctx:
        const = ctx.enter_context(tc.tile_pool(name="const", bufs=1))

        eTa = const.tile([KA, NS], MM_DT)    # [65, 8192]
        chat = const.tile([KA, C], MM_DT)    # [65, 2048]: c^T rows + c2
        csqb = const.tile([D, C], BF16)
        ensb = const.tile([P, T * D], BF16)  # e natural, tile-major
        clab = const.tile([P, T * D], F32)   # gathered centers per row
        cdif = const.tile([P, T * D], BF16)  # e - c[label], bf16
        scrb = const.tile([P, T * D], BF16)  # scratch squares
        labsb = const.tile([P, T], I32)
        ones = const.tile([P, 1], BF16)
        onesf = const.tile([P, 1], F32)
        zaccs = [const.tile([P, C], BF16, name=f"zacc{i}") for i in range(4)]
        e2 = const.tile([P, T], F32)
        d2 = const.tile([P, T], F32)
        dall = const.tile([P, T], F32)
        macc = const.tile([P, T], F32)
        rowtot = const.tile([P, 1], F32)
        zrow = const.tile([P, 1], F32)
        outsb = const.tile([1, 1], F32)

        # labels first: the gpsimd gather stream is gated only on this DMA
        nc.sync.dma_start(labsb[:], labT)
        # c^T lands straight into the matmul rhs rows 0..63
        nc.sync.dma_start(chat[0:D, :], cTb)
        nc.vector.memset(ones[:], 1.0)
        nc.vector.memset(onesf[:], 1.0)
        nc.vector.memset(macc[:], 0.0)
        # all e-side loads issued up front (per-group slices keep the
        # tile framework's subtile deps intact)
        for g in range(NG):
            cs, ce = g * G * P, (g + 1) * G * P
            fs, fe = g * G * D, (g + 1) * G * D
            nc.sync.dma_start(eTa[:, cs:ce], eT[:, cs:ce])
            nc.sync.dma_start(
                ensb[:, fs:fe].rearrange("p (t d) -> p t d", d=D),
                enat[cs:ce, :].rearrange("(t p) d -> p t d", p=P),
            )

        mm_ctx = tc.tile_pool(name="mm", bufs=2, space="PSUM")
        mm_pool = mm_ctx.__enter__()

        # c2 row: csq = (c^T)^2, ones-matmul column sum into psum
        # partition 64, copy across to chat row 64 (same partition).
        c2ps_full = mm_pool.tile([P, C], F32, name="ps", tag="ps")
        for k in range(NB):
            sl = slice(k * FD, (k + 1) * FD)
            nc.vector.tensor_mul(csqb[:, sl], chat[0:D, sl], chat[0:D, sl])
            nc.tensor.matmul(
                c2ps_full[64:65, sl], lhsT=ones[0:D, :], rhs=csqb[:, sl],
                start=True, stop=True,
            )
            c2_i = nc.vector.tensor_copy(chat[D:KA, sl], c2ps_full[64:65, sl])

        dist_pool = ctx.enter_context(tc.tile_pool(name="dist", bufs=6))
        from concourse.tile import add_dep_helper

        z_insts = []
        nh = 0
        NSPLIT = 28
        bounds = [(0, G // 2), (G // 2, G)] + [
            (g * G, (g + 1) * G) for g in range(1, NG)
        ]
        for gi, (ts, te) in enumerate(bounds):
            fs, fe = ts * D, te * D
            for t in range(ts, te):
                nc.gpsimd.indirect_dma_start(
                    out=clab[:, t * D:(t + 1) * D],
                    out_offset=None,
                    in_=cnat,
                    in_offset=bass.IndirectOffsetOnAxis(ap=labsb[:, t:t + 1], axis=0),
                )
            # per-row e2, d2, d
            nc.vector.tensor_mul(scrb[:, fs:fe], ensb[:, fs:fe], ensb[:, fs:fe])
            nc.vector.tensor_reduce(
                e2[:, ts:te], scrb[:, fs:fe].rearrange("p (t d) -> p t d", d=D),
                axis=AX.X, op=OP.add,
            )
            nc.gpsimd.tensor_sub(
                cdif[:, fs:fe], ensb[:, fs:fe], clab[:, fs:fe]
            )
            sub_i = nc.vector.tensor_mul(
                scrb[:, fs:fe], cdif[:, fs:fe], cdif[:, fs:fe]
            )
            if gi >= 2:
                # keep the gather-gated d2 chain BEHIND the previous group's
                # main DVE ops in the scheduled stream
                add_dep_helper(sub_i.ins, z_insts[ts - 3].ins, sync=False,
                               reason="hold d2 chain behind prior group")
            elif gi == 1:
                add_dep_helper(sub_i.ins, z_insts[1].ins, sync=False,
                               reason="hold d2 chain behind prior group")
            else:
                add_dep_helper(sub_i.ins, c2_i.ins, sync=False,
                               reason="hold g0 d2 chain behind chat build")
            nc.vector.tensor_reduce(
                d2[:, ts:te], scrb[:, fs:fe].rearrange("p (t d) -> p t d", d=D),
                axis=AX.X, op=OP.add,
            )
            nc.scalar.activation(dall[:, ts:te], d2[:, ts:te], AF.Sqrt)

            # main tiles of this group
            for t in range(ts, te):
                ps = mm_pool.tile([P, C], F32, name="ps")
                lhsT = eTa[:, t * P:(t + 1) * P]
                for k in range(NB):
                    nc.tensor.matmul(
                        ps[:, k * FD:(k + 1) * FD],
                        lhsT=lhsT,
                        rhs=chat[:, k * FD:(k + 1) * FD],
                        start=True, stop=True,
                    )
                xz = dist_pool.tile([P, C], BF16, name="dist")
                nc.scalar.activation(
                    xz[:], ps[:], AF.Sqrt,
                    bias=e2[:, t:t + 1], scale=1.0,
                )
                zi = nc.vector.tensor_scalar(
                    out=xz[:], in0=xz[:],
                    scalar1=dall[:, t:t + 1], scalar2=dall[:, t:t + 1],
                    op0=OP.min, op1=OP.subtract,
                )
                z_insts.append(zi)
                if t in A_TILES:
                    nc.scalar.activation(
                        xz[:], xz[:], AF.Identity,
                        accum_out=macc[:, t:t + 1],
                    )
                else:
                    if nh == NSPLIT:
                        nc.vector.tensor_add(zaccs[0][:], zaccs[0][:],
                                             zaccs[1][:])
                    if nh < NSPLIT:
                        za = zaccs[nh % 2]
                        first = nh < 2
                    else:
                        za = zaccs[2 + nh % 2]
                        first = nh - NSPLIT < 2
                    if first:
                        nc.vector.tensor_copy(za[:], xz[:])
                    else:
                        nc.vector.tensor_add(za[:], za[:], xz[:])
                    nh += 1

        mm_ctx.__exit__(None, None, None)

        # loss rows = -(sum_j macc + sum_j sum-of-zaccs); merge accs pairwise
        nc.vector.tensor_add(zaccs[2][:], zaccs[2][:], zaccs[3][:])
        nc.vector.tensor_add(zaccs[0][:], zaccs[0][:], zaccs[2][:])
        nc.vector.tensor_reduce(rowtot[:], macc[:], axis=AX.X, op=OP.add)
        nc.vector.tensor_reduce(zrow[:], zaccs[0][:], axis=AX.X, op=OP.add)
        nc.vector.tensor_add(rowtot[:], rowtot[:], zrow[:])
        nc.vector.tensor_scalar_mul(rowtot[:], rowtot[:], -1.0)
        with tc.tile_pool(name="fin", bufs=1, space="PSUM") as finp:
            fin = finp.tile([1, 1], F32)
            nc.tensor.matmul(fin[:], lhsT=rowtot[:], rhs=onesf[:],
                             start=True, stop=True)
            nc.scalar.copy(outsb[:], fin[:])
        nc.sync.dma_start(out, outsb[:])


_NC_CACHE = {}


def build_nc():
    if "nc" in _NC_CACHE:
        return _NC_CACHE["nc"]
    nc = bacc.Bacc(
        "TRN2", target_bir_lowering=False, debug=False, enable_asserts=False
    )
    eT = nc.dram_tensor("eT", [KA, NS], MM_DT, kind="ExternalInput").ap()
    enat = nc.dram_tensor("enat", [NS, D], BF16, kind="ExternalInput").ap()
    labT = nc.dram_tensor("labT", [P, T], I32, kind="ExternalInput").ap()
    cTb = nc.dram_tensor("cTb", [D, C], BF16, kind="ExternalInput").ap()
    cnat = nc.dram_tensor("cnat", [C, D], F32, kind="ExternalInput").ap()
    out = nc.dram_tensor("out", [1, 1], F32, kind="ExternalOutput").ap()
    with nc.allow_low_precision(reason="bf16 distance pipeline"):
        with tile.TileContext(nc) as tc:
            _body(tc, out, eT, enat, labT, cTb, cnat)
    nc.compile()
    _NC_CACHE["nc"] = nc
    return nc


def make_in_maps(embeddings, centers, labels):
    e = np.ascontiguousarray(np.asarray(embeddings, dtype=np.float32))
    c = np.ascontiguousarray(np.asarray(centers, dtype=np.float32))
    lab = np.asarray(labels).astype(np.int32)
    assert e.shape == (N, D) and c.shape == (C, D) and lab.shape == (N,)
    cTb = np.ascontiguousarray(c.T).astype(ml_dtypes.bfloat16)
    in_maps = []
    for core in range(NCORES):
        es = e[core * NS:(core + 1) * NS]
        ls = lab[core * NS:(core + 1) * NS]
        eT65 = np.ones((KA, NS), np.float32)
        eT65[0:D] = -2.0 * es.T
        eT65 = eT65.astype(ml_dtypes.bfloat16)
        in_maps.append({
            "eT": eT65,
            "enat": np.ascontiguousarray(es.astype(ml_dtypes.bfloat16)),
            "labT": np.ascontiguousarray(ls.reshape(T, P).T),
            "cTb": cTb,
            "cnat": c,
        })
    return in_maps


def run(embeddings, centers, labels, **kw):
    nc = build_nc()
    in_maps = make_in_maps(embeddings, centers, labels)
    res = run_bass_kernel_spmd(nc, in_maps, core_ids=list(range(NCORES)), **kw)
    total = float(sum(float(r["out"][0, 0]) for r in res.results))
    return np.float32(total), res


def kernel(embeddings, centers, labels):
    val, _ = run(embeddings, centers, labels)
    return val
